# revision 33
# baseline (speedup 1.0000x reference)
"""Trainium2 Bass kernel for nn_Kernel_Conv_83554293776951.

Reference computation (batch-as-groups dynamic conv):
    y      = relu(W1 @ x + b1)                       per (b, hw)
    weight = W2 @ y + b2                             [b, O*C, hw]
    out[b,o] = sum_{c,hw} x[b,c,hw] * weight[b,(o,c),hw]

Algebraic rewrite (8x fewer FLOPs): contract hw first into a per-sample
Gram tensor G[b,c,k] = sum_hw x[b,c,hw]*y[b,k,hw], then
    out[b,o] = sum_{c,k} G[b,c,k]*W2[(o,c),k] + sum_c b2[(o,c)]*xs[b,c]
with xs[b,c] = sum_hw x[b,c,hw].  The attention branch of the reference
is dead code and is skipped.

Sharding: the contraction dim c (256) is split across 8 cores (32 each).
Each core streams its W2 slice and produces a partial [32, 512] output;
partials are summed on the host.  The kernel is DMA-bound on the W2
stream, so W2 is quantized host-side to fp8 e3m4 (x32 scale, folded
back via a 1/32 on G) halving the stream to 4.2MB/core; the matmul runs
mixed bf16(G) x fp8(W2).  W2 is stored in the exact SBUF layout so each
DMA moves 4KB-contiguous per-partition blocks, and the 64 accumulation
chunks run col-tiled 4-wide (tile_position) so four M=32 matmuls share
the PE array concurrently.

Scheduling notes (from perfetto traces): the packed small-tensor load
is split in two and interleaved ahead of the W2 flood on the same
HWDGE ring (FIFO) so the y/Gram critical path starts ~9.5us; PSUM/SBUF
tiles for the y phase are split per group because Tile tracks deps at
whole-tile granularity (single tiles serialize PE vs DVE/ACT); the ACT
lookup-table load is pulled early with a dummy op; the output tail is
pipelined in two o-halves across DVE/ACT and both DMA rings.
"""

import sys

for _p in ("/opt/trn_rl_repo",):
    if _p not in sys.path:
        sys.path.insert(0, _p)

import numpy as np
import ml_dtypes

B = 32          # batch
C = 256         # in channels (contraction "c")
KD = 256        # hidden dim of y (contraction "k")
O = 512         # out channels
HW = 16         # spatial 4x4
NCORES = 8
CS = C // NCORES          # 32 c-channels per core
NCHUNK = CS * (KD // 128)  # 64 contraction chunks of 128 per core
NTILE = 8                  # W2 DMA tiles (8 chunks each)
W2SCALE = 32.0             # fp8 e3m4 scale; 1/32 folded into G

# packed small-tensor buffer (single DMA, first on the sync ring)
PK_XT = 0          # [2, 512]  x as [c%128, cc, (b,hw)]
PK_W1T = 1024      # [2, 256]  W1^T as [c%128, cc, k]
PK_B1 = 1536       # [256]     b1 broadcast to all partitions
PK_XBD = 1792      # [4, 256]  block-diag x for Gram
PK_B2 = 2816       # [2, 512]  b2 rows for this core's c-slice (else 0)
PK_SEL = 3840      # [32]      sel[p, m] = (p % 32 == m)
PK_TOT = 3872

_CACHE = {}


def _build_nc():
    import concourse.bass as bass  # noqa: F401
    from concourse import bacc
    import concourse.mybir as mybir
    import concourse.tile as tile

    f32 = mybir.dt.float32
    bf16 = mybir.dt.bfloat16
    f8 = mybir.dt.float8e3

    nc = bacc.Bacc(None, target_bir_lowering=False)

    with tile.TileContext(nc) as tc:
        with tc.tile_pool(name="dram", bufs=1, space="DRAM") as dram:
            pk_d = dram.tile([128, PK_TOT], bf16, kind="ExternalInput", uniquify=False, name="pk")
            w2_d = dram.tile([128, NCHUNK, O], f8, kind="ExternalInput", uniquify=False, name="w2")
            out_d = dram.tile([B, O], f32, kind="ExternalOutput", uniquify=False, name="out")

            from contextlib import ExitStack

            stack = ExitStack()
            consts = stack.enter_context(tc.tile_pool(name="consts", bufs=1))
            w2pool = stack.enter_context(tc.tile_pool(name="w2pool", bufs=1))
            psum_y = stack.enter_context(tc.tile_pool(name="psum_y", bufs=1, space="PSUM"))
            psum_g = stack.enter_context(tc.tile_pool(name="psum_g", bufs=1, space="PSUM"))
            psum_o = stack.enter_context(tc.tile_pool(name="psum_o", bufs=1, space="PSUM"))

            # ---- DMA plan.  DMAs issued on one HWDGE ring do NOT serialize
            # data-wise: the SDMA engines round-robin packets of every
            # in-flight DMA.  The ~0.65us per-issue cost on the sync queue is
            # used as a natural stagger for the W2 tiles; pk1 is issued first
            # (sync) and pk2 first on the ACT queue so the y/Gram inputs only
            # ever share bandwidth with one young W2 tile. ----
            pk_sb = consts.tile([128, PK_TOT], bf16)
            nc.sync.dma_start(out=pk_sb[:], in_=pk_d[:, :])
            # W2 issues split across both HWDGE queues: each issue occupies
            # its queue ~0.65us, and a tile only starts streaming once
            # issued -- one queue would push the last tile's start past 12us
            w2_sb = []
            for t in range(NTILE):
                w = w2pool.tile([128, NTILE, O], f8, name=f"w2sb{t}")
                eng = nc.sync if t < NTILE // 2 else nc.scalar
                eng.dma_start(out=w[:], in_=w2_d[:, t * NTILE:(t + 1) * NTILE, :])
                w2_sb.append(w)

            xT_v = pk_sb[:, PK_XT:PK_XT + 1024].rearrange("p (c f) -> p c f", c=2)
            w1t_v = pk_sb[:, PK_W1T:PK_W1T + 512].rearrange("p (c k) -> p c k", c=2)
            b1row_v = pk_sb[0:1, PK_B1:PK_B1 + KD]
            xbd_v = pk_sb[:, PK_XBD:PK_XBD + 1024].rearrange("p (g f) -> p g f", g=4)
            b2fp_v = pk_sb[:, PK_B2:PK_B2 + 2 * O].rearrange("p (c o) -> p c o", c=2)
            selm_v = pk_sb[:, PK_SEL:PK_SEL + 32]

            ones_sb = consts.tile([1, 128], bf16)
            nc.gpsimd.memset(ones_sb[:], 1.0)
            # dummy op pulls the 1.3us ACT table load off the critical path
            act_warm = consts.tile([1, 128], bf16)
            nc.scalar.activation(act_warm[:], ones_sb[:], mybir.ActivationFunctionType.Relu)

            # ---- PE warm-up: ~6 throwaway matmuls while waiting for pk1 so
            # the HAM clock-gate releases (1.2 -> 2.4 GHz) before real work;
            # they write into `ops`, which the round-0 start=True clears ----
            # HAM releases the PE clock-gate (1.2 -> 2.4 GHz) only after ~4us
            # of saturated FULL-ARRAY activity (K=1 matmuls do not count);
            # run ~4us of K=128 no-dep matmuls while pk1 is still in flight
            # so the y/Gram phase and the big stream run at 2.4 GHz.
            warm_sb = consts.tile([128, O], bf16)
            nc.vector.memset(warm_sb[:], 0.001)
            ops = psum_o.tile([128, O], f32)
            for _ in range(8):
                nc.tensor.matmul(
                    ops[:, :], lhsT=warm_sb[:, 0:128], rhs=warm_sb[:],
                    start=True, stop=True,
                )

            # ---- step 1: y = relu(W1 @ x + b1) in [(b,hw) part, k] layout.
            # b1 enters as a rank-1 matmul chunk (ones^T @ b1row).  yps is
            # ping-ponged over two PSUM tiles and y_sb split per group so
            # Tile's whole-tile dep tracking cannot serialize PE vs DVE/ACT.
            yps = [psum_y.tile([128, 2, KD], f32, name=f"yps{i}") for i in range(2)]
            y_sb = [consts.tile([128, KD], bf16, name=f"ysb{g}") for g in range(4)]
            for g in range(4):
                dst = yps[g % 2][:, g // 2, :]
                for cc in range(2):
                    nc.tensor.matmul(
                        dst,
                        lhsT=xT_v[:, cc, g * 128:(g + 1) * 128],
                        rhs=w1t_v[:, cc, :],
                        start=(cc == 0),
                        stop=False,
                    )
                nc.tensor.matmul(
                    dst, lhsT=ones_sb[:], rhs=b1row_v, start=False, stop=True,
                )
                if g % 2 == 0:
                    nc.vector.tensor_scalar_max(y_sb[g][:], dst, 0.0)
                else:
                    nc.scalar.activation(
                        y_sb[g][:], dst, mybir.ActivationFunctionType.Relu,
                    )

            # ---- step 2: Gram  G[k, b*32+c~] = sum_hw y[(b,hw),k] x[b,c~,hw],
            # with the 1/W2SCALE fold applied on the PSUM->SBUF cast ----
            # gps split into 4 one-bank tiles (kh x group-half) so each
            # PSUM->SBUF cast can start as soon as its two gram MMs finish
            gps = [
                [psum_g.tile([128, 512], f32, name=f"gps{kh}{h}") for h in range(2)]
                for kh in range(2)
            ]
            g_sb = [consts.tile([128, B * CS], bf16, name=f"gsb{kh}") for kh in range(2)]
            for kh in range(2):
                for g in range(4):
                    nc.tensor.matmul(
                        gps[kh][g // 2][:, (g % 2) * 256:(g % 2 + 1) * 256],
                        lhsT=y_sb[g][:, kh * 128:(kh + 1) * 128],
                        rhs=xbd_v[:, g, :],
                        start=True,
                        stop=True,
                    )
                for h in range(2):
                    dst = g_sb[kh][:, h * 512:(h + 1) * 512]
                    if h == 0:
                        nc.vector.tensor_scalar_mul(dst, gps[kh][h][:], 1.0 / W2SCALE)
                    else:
                        nc.scalar.mul(dst, gps[kh][h][:], 1.0 / W2SCALE)

            # ---- xs[c, b] = sum_hw x[b, c, hw]  (for the b2 bias term);
            # one reduce -- the scheduler hoists it into the DVE idle window
            # before the relus ----
            xs_sb = consts.tile([128, 2, B], bf16)
            with nc.allow_low_precision(reason="16-elem sum into bf16; feeds small bias term"):
                nc.vector.tensor_reduce(
                    out=xs_sb[:].rearrange("p c b -> p (c b)"),
                    in_=pk_sb[:, PK_XT:PK_XT + 1024].rearrange("p (f h) -> p f h", h=HW),
                    axis=mybir.AxisListType.X,
                    op=mybir.AluOpType.add,
                )

            # keep the PE saturated through the G-copy gap: ~2us of idle
            # here re-throttles the clock-gate and the stream starts cold
            for _ in range(5):
                nc.tensor.matmul(
                    ops[:, :], lhsT=warm_sb[:, 0:128], rhs=warm_sb[:],
                    start=True, stop=True,
                )

            # ---- step 3: big stream, col-tiled 4-wide over psum partitions ----
            for r in range(NCHUNK // 4):
                for j in range(4):
                    ch = 4 * r + j
                    ct, kh = ch % 32, ch // 32
                    nc.tensor.matmul(
                        ops[32 * j:32 * (j + 1), :],
                        lhsT=g_sb[kh].rearrange("p (b c) -> p c b", c=CS)[:, ct, :],
                        rhs=w2_sb[ch // 8][:, ch % 8, :],
                        start=(r == 0),
                        stop=(r == NCHUNK // 4 - 1),
                        tile_position=(0, 32 * j),
                    )
                if r == 4:
                    # bias chunks join col-group 0 mid-stream, off the tail
                    # (b2fp rows are zero off-slice; one matmul per cc half
                    # since the program is shared SPMD)
                    for cc in range(2):
                        nc.tensor.matmul(
                            ops[0:32, :],
                            lhsT=xs_sb[:, cc, :],
                            rhs=b2fp_v[:, cc, :],
                            start=False,
                            stop=False,
                            tile_position=(0, 0),
                        )

            # ---- tail, pipelined in two o-halves: PSUM->SBUF bf16 cast,
            # col-group reduce via sel matmul, fp32 copy-out, DMA out on
            # both rings ----
            o_sb = consts.tile([128, O], bf16)
            ops2 = psum_o.tile([B, O], f32)
            out_sb = consts.tile([B, O], f32)
            HO = O // 2
            for h in range(2):
                sl = slice(h * HO, (h + 1) * HO)
                if h == 0:
                    nc.vector.tensor_copy(out=o_sb[:, sl], in_=ops[:, sl])
                else:
                    nc.scalar.copy(o_sb[:, sl], ops[:, sl])
                nc.tensor.matmul(
                    ops2[:, sl], lhsT=selm_v, rhs=o_sb[:, sl],
                    start=True, stop=True,
                )
                if h == 0:
                    nc.vector.tensor_copy(out=out_sb[:, sl], in_=ops2[:, sl])
                    nc.scalar.dma_start(out=out_d[:, sl], in_=out_sb[:, sl])
                else:
                    nc.scalar.copy(out_sb[:, sl], ops2[:, sl])
                    nc.sync.dma_start(out=out_d[:, sl], in_=out_sb[:, sl])

            stack.close()

    nc.compile()
    return nc


def _prep_in_maps(x, W1, b1, W2, b2):
    bf = ml_dtypes.bfloat16
    f8 = ml_dtypes.float8_e3m4
    x = np.ascontiguousarray(np.asarray(x, dtype=np.float32)).reshape(B, C, HW)
    W1 = np.asarray(W1, dtype=np.float32)
    b1 = np.asarray(b1, dtype=np.float32)
    W2 = np.asarray(W2, dtype=np.float32)
    b2 = np.asarray(b2, dtype=np.float32)

    # shared part of pk
    pk0 = np.zeros((128, PK_TOT), dtype=bf)
    xT = x.transpose(1, 0, 2).reshape(2, 128, B * HW).transpose(1, 0, 2)
    pk0[:, PK_XT:PK_XT + 1024] = xT.reshape(128, 1024).astype(bf)
    w1t = W1.T.reshape(2, 128, KD).transpose(1, 0, 2)
    pk0[:, PK_W1T:PK_W1T + 512] = w1t.reshape(128, 512).astype(bf)
    pk0[:, PK_B1:PK_B1 + KD] = np.broadcast_to(b1.astype(bf), (128, KD))
    sel = (np.arange(128)[:, None] % 32 == np.arange(32)[None, :])
    pk0[:, PK_SEL:PK_SEL + 32] = sel.astype(bf)

    W2r = W2.reshape(O, C, KD)
    b2r = b2.reshape(O, C)

    in_maps = []
    for i in range(NCORES):
        c0 = i * CS
        pk = pk0.copy()
        # block-diag x: rows (b%8)*16+hw, group b//8, cols (b%8)*32+c~
        xbd = np.zeros((128, 4, 256), dtype=bf)
        for b in range(B):
            g, jb = b // 8, b % 8
            xbd[16 * jb:16 * (jb + 1), g, CS * jb:CS * (jb + 1)] = (
                x[b, c0:c0 + CS, :].T.astype(bf)
            )
        pk[:, PK_XBD:PK_XBD + 1024] = xbd.reshape(128, 1024)
        p0, ccH = c0 % 128, c0 // 128
        pk[p0:p0 + CS, PK_B2 + ccH * O:PK_B2 + (ccH + 1) * O] = (
            b2r[:, c0:c0 + CS].T.astype(bf)
        )
        # w2 stream: [p, ch, o] with ch = kh*32 + ct, value 32*W2[(o,c0+ct), kh*128+p]
        w2s = (W2r[:, c0:c0 + CS, :] * W2SCALE).transpose(2, 1, 0)  # [k, ct, o]
        w2s = w2s.reshape(2, 128, CS, O).transpose(1, 0, 2, 3)      # [p, kh, ct, o]
        w2q = np.ascontiguousarray(w2s.reshape(128, NCHUNK, O)).astype(f8)
        in_maps.append({"pk": pk, "w2": w2q})
    return in_maps


def kernel(x, W1, b1, W2, b2, Wa=None, ba=None, **_unused):
    from concourse.bass_utils import run_bass_kernel_spmd

    if "nc" not in _CACHE:
        _CACHE["nc"] = _build_nc()
    nc = _CACHE["nc"]

    in_maps = _prep_in_maps(x, W1, b1, W2, b2)
    res = run_bass_kernel_spmd(nc, in_maps, core_ids=list(range(NCORES)))
    partials = [r["out"].astype(np.float64) for r in res.results]
    out = np.sum(partials, axis=0).astype(np.float32)
    return out.reshape(B, O, 1, 1)


# revision 34
# speedup vs baseline: 1.1741x; 1.1741x over previous
"""Trainium2 Bass kernel for nn_Kernel_Conv_83554293776951.

Reference computation (batch-as-groups dynamic conv):
    y      = relu(W1 @ x + b1)                       per (b, hw)
    weight = W2 @ y + b2                             [b, O*C, hw]
    out[b,o] = sum_{c,hw} x[b,c,hw] * weight[b,(o,c),hw]

Algebraic rewrite (8x fewer FLOPs): contract hw first into a per-sample
Gram tensor G[b,c,k] = sum_hw x[b,c,hw]*y[b,k,hw], then
    out[b,o] = sum_{c,k} G[b,c,k]*W2[(o,c),k] + sum_c b2[(o,c)]*xs[b,c]
with xs[b,c] = sum_hw x[b,c,hw].  The attention branch of the reference
is dead code and is skipped.

Sharding: the contraction dim c (256) is split across 8 cores (32 each).
Each core streams its W2 slice and produces a partial [32, 512] output;
partials are summed on the host.  The kernel is DMA-bound on the W2
stream, so W2 is quantized host-side to fp8 e3m4 (x32 scale, folded
back via a 1/32 on G) halving the stream to 4.2MB/core; the matmul runs
mixed bf16(G) x fp8(W2).  W2 is stored in the exact SBUF layout so each
DMA moves 4KB-contiguous per-partition blocks, and the 64 accumulation
chunks run col-tiled 4-wide (tile_position) so four M=32 matmuls share
the PE array concurrently.

Scheduling notes (from perfetto traces): the packed small-tensor load
is split in two and interleaved ahead of the W2 flood on the same
HWDGE ring (FIFO) so the y/Gram critical path starts ~9.5us; PSUM/SBUF
tiles for the y phase are split per group because Tile tracks deps at
whole-tile granularity (single tiles serialize PE vs DVE/ACT); the ACT
lookup-table load is pulled early with a dummy op; the output tail is
pipelined in two o-halves across DVE/ACT and both DMA rings.
"""

import sys

for _p in ("/opt/trn_rl_repo",):
    if _p not in sys.path:
        sys.path.insert(0, _p)

import numpy as np
import ml_dtypes

B = 32          # batch
C = 256         # in channels (contraction "c")
KD = 256        # hidden dim of y (contraction "k")
O = 512         # out channels
HW = 16         # spatial 4x4
NCORES = 8
CS = C // NCORES          # 32 c-channels per core
NCHUNK = CS * (KD // 128)  # 64 contraction chunks of 128 per core
NTILE = 4                  # W2 DMA tiles (16 chunks each)
W2SCALE = 32.0             # fp8 e3m4 scale; 1/32 folded into G

# packed small-tensor buffer (single DMA, first on the sync ring)
PK_XT = 0          # [2, 512]  x as [c%128, cc, (b,hw)]
PK_W1T = 1024      # [2, 256]  W1^T as [c%128, cc, k]
PK_B1 = 1536       # [256]     b1 broadcast to all partitions
PK_XBD = 1792      # [4, 256]  block-diag x for Gram
PK_B2 = 2816       # [2, 512]  b2 rows for this core's c-slice (else 0)
PK_SEL = 3840      # [32]      sel[p, m] = (p % 32 == m)
PK_TOT = 3872

_CACHE = {}


def _build_nc():
    import concourse.bass as bass  # noqa: F401
    from concourse import bacc
    import concourse.mybir as mybir
    import concourse.tile as tile

    f32 = mybir.dt.float32
    bf16 = mybir.dt.bfloat16
    f8 = mybir.dt.float8e3

    nc = bacc.Bacc(None, target_bir_lowering=False)

    with tile.TileContext(nc) as tc:
        with tc.tile_pool(name="dram", bufs=1, space="DRAM") as dram:
            pk_d = dram.tile([128, PK_TOT], bf16, kind="ExternalInput", uniquify=False, name="pk")
            w2_d = dram.tile([128, NCHUNK, O], f8, kind="ExternalInput", uniquify=False, name="w2")
            out_d = dram.tile([B, O], f32, kind="ExternalOutput", uniquify=False, name="out")

            from contextlib import ExitStack

            stack = ExitStack()
            consts = stack.enter_context(tc.tile_pool(name="consts", bufs=1))
            w2pool = stack.enter_context(tc.tile_pool(name="w2pool", bufs=1))
            psum_y = stack.enter_context(tc.tile_pool(name="psum_y", bufs=1, space="PSUM"))
            psum_g = stack.enter_context(tc.tile_pool(name="psum_g", bufs=1, space="PSUM"))
            psum_o = stack.enter_context(tc.tile_pool(name="psum_o", bufs=1, space="PSUM"))

            # ---- DMA plan.  DMAs issued on one HWDGE ring do NOT serialize
            # data-wise: the SDMA engines round-robin packets of every
            # in-flight DMA.  The ~0.65us per-issue cost on the sync queue is
            # used as a natural stagger for the W2 tiles; pk1 is issued first
            # (sync) and pk2 first on the ACT queue so the y/Gram inputs only
            # ever share bandwidth with one young W2 tile. ----
            pk_sb = consts.tile([128, PK_TOT], bf16)
            nc.sync.dma_start(out=pk_sb[:], in_=pk_d[:, :])
            # 4 W2 tiles of 16 chunks: 8KB-contiguous per-partition
            # descriptors, few issues (all done by ~10us), and pk+4+2out
            # DMAs fit the 8 semaphore lanes with zero reuse stalls
            TCH = NCHUNK // NTILE
            w2_sb = []
            for t in range(NTILE):
                w = w2pool.tile([128, TCH, O], f8, name=f"w2sb{t}")
                nc.sync.dma_start(out=w[:], in_=w2_d[:, t * TCH:(t + 1) * TCH, :])
                w2_sb.append(w)

            xT_v = pk_sb[:, PK_XT:PK_XT + 1024].rearrange("p (c f) -> p c f", c=2)
            w1t_v = pk_sb[:, PK_W1T:PK_W1T + 512].rearrange("p (c k) -> p c k", c=2)
            b1row_v = pk_sb[0:1, PK_B1:PK_B1 + KD]
            xbd_v = pk_sb[:, PK_XBD:PK_XBD + 1024].rearrange("p (g f) -> p g f", g=4)
            b2fp_v = pk_sb[:, PK_B2:PK_B2 + 2 * O].rearrange("p (c o) -> p c o", c=2)
            selm_v = pk_sb[:, PK_SEL:PK_SEL + 32]

            ones_sb = consts.tile([1, 128], bf16)
            nc.gpsimd.memset(ones_sb[:], 1.0)
            # dummy op pulls the 1.3us ACT table load off the critical path
            act_warm = consts.tile([1, 128], bf16)
            nc.scalar.activation(act_warm[:], ones_sb[:], mybir.ActivationFunctionType.Relu)

            # ---- PE warm-up: ~6 throwaway matmuls while waiting for pk1 so
            # the HAM clock-gate releases (1.2 -> 2.4 GHz) before real work;
            # they write into `ops`, which the round-0 start=True clears ----
            # HAM releases the PE clock-gate (1.2 -> 2.4 GHz) only after ~4us
            # of saturated FULL-ARRAY activity (K=1 matmuls do not count);
            # run ~4us of K=128 no-dep matmuls while pk1 is still in flight
            # so the y/Gram phase and the big stream run at 2.4 GHz.
            warm_sb = consts.tile([128, O], bf16)
            nc.vector.memset(warm_sb[:], 0.001)
            ops = psum_o.tile([128, O], f32)
            for _ in range(8):
                nc.tensor.matmul(
                    ops[:, :], lhsT=warm_sb[:, 0:128], rhs=warm_sb[:],
                    start=True, stop=True,
                )

            # ---- step 1: y = relu(W1 @ x + b1) in [(b,hw) part, k] layout.
            # b1 enters as a rank-1 matmul chunk (ones^T @ b1row).  yps is
            # ping-ponged over two PSUM tiles and y_sb split per group so
            # Tile's whole-tile dep tracking cannot serialize PE vs DVE/ACT.
            yps = [psum_y.tile([128, 2, KD], f32, name=f"yps{i}") for i in range(2)]
            y_sb = [consts.tile([128, KD], bf16, name=f"ysb{g}") for g in range(4)]
            for g in range(4):
                dst = yps[g % 2][:, g // 2, :]
                for cc in range(2):
                    nc.tensor.matmul(
                        dst,
                        lhsT=xT_v[:, cc, g * 128:(g + 1) * 128],
                        rhs=w1t_v[:, cc, :],
                        start=(cc == 0),
                        stop=False,
                    )
                nc.tensor.matmul(
                    dst, lhsT=ones_sb[:], rhs=b1row_v, start=False, stop=True,
                )
                if g % 2 == 0:
                    nc.vector.tensor_scalar_max(y_sb[g][:], dst, 0.0)
                else:
                    nc.scalar.activation(
                        y_sb[g][:], dst, mybir.ActivationFunctionType.Relu,
                    )

            # ---- step 2: Gram  G[k, b*32+c~] = sum_hw y[(b,hw),k] x[b,c~,hw],
            # with the 1/W2SCALE fold applied on the PSUM->SBUF cast ----
            # gps split into 4 one-bank tiles (kh x group-half) so each
            # PSUM->SBUF cast can start as soon as its two gram MMs finish
            gps = [
                [psum_g.tile([128, 512], f32, name=f"gps{kh}{h}") for h in range(2)]
                for kh in range(2)
            ]
            g_sb = [consts.tile([128, B * CS], bf16, name=f"gsb{kh}") for kh in range(2)]
            for kh in range(2):
                for g in range(4):
                    nc.tensor.matmul(
                        gps[kh][g // 2][:, (g % 2) * 256:(g % 2 + 1) * 256],
                        lhsT=y_sb[g][:, kh * 128:(kh + 1) * 128],
                        rhs=xbd_v[:, g, :],
                        start=True,
                        stop=True,
                    )
                for h in range(2):
                    dst = g_sb[kh][:, h * 512:(h + 1) * 512]
                    if h == 0:
                        nc.vector.tensor_scalar_mul(dst, gps[kh][h][:], 1.0 / W2SCALE)
                    else:
                        nc.scalar.mul(dst, gps[kh][h][:], 1.0 / W2SCALE)

            # ---- xs[c, b] = sum_hw x[b, c, hw]  (for the b2 bias term);
            # one reduce -- the scheduler hoists it into the DVE idle window
            # before the relus ----
            xs_sb = consts.tile([128, 2, B], bf16)
            with nc.allow_low_precision(reason="16-elem sum into bf16; feeds small bias term"):
                nc.vector.tensor_reduce(
                    out=xs_sb[:].rearrange("p c b -> p (c b)"),
                    in_=pk_sb[:, PK_XT:PK_XT + 1024].rearrange("p (f h) -> p f h", h=HW),
                    axis=mybir.AxisListType.X,
                    op=mybir.AluOpType.add,
                )

            # keep the PE saturated through the G-copy gap: ~2us of idle
            # here re-throttles the clock-gate and the stream starts cold
            for _ in range(5):
                nc.tensor.matmul(
                    ops[:, :], lhsT=warm_sb[:, 0:128], rhs=warm_sb[:],
                    start=True, stop=True,
                )

            # ---- step 3: big stream, col-tiled 4-wide over psum partitions ----
            for r in range(NCHUNK // 4):
                for j in range(4):
                    ch = 4 * r + j
                    ct, kh = ch % 32, ch // 32
                    nc.tensor.matmul(
                        ops[32 * j:32 * (j + 1), :],
                        lhsT=g_sb[kh].rearrange("p (b c) -> p c b", c=CS)[:, ct, :],
                        rhs=w2_sb[ch // TCH][:, ch % TCH, :],
                        start=(r == 0),
                        stop=(r == NCHUNK // 4 - 1),
                        tile_position=(0, 32 * j),
                    )
                if r == 4:
                    # bias chunks join col-group 0 mid-stream, off the tail
                    # (b2fp rows are zero off-slice; one matmul per cc half
                    # since the program is shared SPMD)
                    for cc in range(2):
                        nc.tensor.matmul(
                            ops[0:32, :],
                            lhsT=xs_sb[:, cc, :],
                            rhs=b2fp_v[:, cc, :],
                            start=False,
                            stop=False,
                            tile_position=(0, 0),
                        )
                if r % 4 == 3 and r < NCHUNK // 4 - 1:
                    # tile boundary: the PE waits ~1.5us for the next W2
                    # tile; keep the clock-gate open with throwaway matmuls
                    # into the dead gps bank (its content was copied out
                    # before round 0)
                    for _ in range(2):
                        nc.tensor.matmul(
                            gps[0][0][:, :], lhsT=warm_sb[:, 0:128],
                            rhs=warm_sb[:], start=True, stop=True,
                        )

            # ---- tail, pipelined in two o-halves: PSUM->SBUF bf16 cast,
            # col-group reduce via sel matmul, fp32 copy-out, DMA out on
            # both rings ----
            o_sb = consts.tile([128, O], bf16)
            ops2 = psum_o.tile([B, O], f32)
            out_sb = consts.tile([B, O], f32)
            HO = O // 2
            for h in range(2):
                sl = slice(h * HO, (h + 1) * HO)
                if h == 0:
                    nc.vector.tensor_copy(out=o_sb[:, sl], in_=ops[:, sl])
                else:
                    nc.scalar.copy(o_sb[:, sl], ops[:, sl])
                nc.tensor.matmul(
                    ops2[:, sl], lhsT=selm_v, rhs=o_sb[:, sl],
                    start=True, stop=True,
                )
                if h == 0:
                    nc.vector.tensor_copy(out=out_sb[:, sl], in_=ops2[:, sl])
                    nc.scalar.dma_start(out=out_d[:, sl], in_=out_sb[:, sl])
                else:
                    nc.scalar.copy(out_sb[:, sl], ops2[:, sl])
                    nc.sync.dma_start(out=out_d[:, sl], in_=out_sb[:, sl])

            stack.close()

    nc.compile()
    return nc


def _prep_in_maps(x, W1, b1, W2, b2):
    bf = ml_dtypes.bfloat16
    f8 = ml_dtypes.float8_e3m4
    x = np.ascontiguousarray(np.asarray(x, dtype=np.float32)).reshape(B, C, HW)
    W1 = np.asarray(W1, dtype=np.float32)
    b1 = np.asarray(b1, dtype=np.float32)
    W2 = np.asarray(W2, dtype=np.float32)
    b2 = np.asarray(b2, dtype=np.float32)

    # shared part of pk
    pk0 = np.zeros((128, PK_TOT), dtype=bf)
    xT = x.transpose(1, 0, 2).reshape(2, 128, B * HW).transpose(1, 0, 2)
    pk0[:, PK_XT:PK_XT + 1024] = xT.reshape(128, 1024).astype(bf)
    w1t = W1.T.reshape(2, 128, KD).transpose(1, 0, 2)
    pk0[:, PK_W1T:PK_W1T + 512] = w1t.reshape(128, 512).astype(bf)
    pk0[:, PK_B1:PK_B1 + KD] = np.broadcast_to(b1.astype(bf), (128, KD))
    sel = (np.arange(128)[:, None] % 32 == np.arange(32)[None, :])
    pk0[:, PK_SEL:PK_SEL + 32] = sel.astype(bf)

    W2r = W2.reshape(O, C, KD)
    b2r = b2.reshape(O, C)

    in_maps = []
    for i in range(NCORES):
        c0 = i * CS
        pk = pk0.copy()
        # block-diag x: rows (b%8)*16+hw, group b//8, cols (b%8)*32+c~
        xbd = np.zeros((128, 4, 256), dtype=bf)
        for b in range(B):
            g, jb = b // 8, b % 8
            xbd[16 * jb:16 * (jb + 1), g, CS * jb:CS * (jb + 1)] = (
                x[b, c0:c0 + CS, :].T.astype(bf)
            )
        pk[:, PK_XBD:PK_XBD + 1024] = xbd.reshape(128, 1024)
        p0, ccH = c0 % 128, c0 // 128
        pk[p0:p0 + CS, PK_B2 + ccH * O:PK_B2 + (ccH + 1) * O] = (
            b2r[:, c0:c0 + CS].T.astype(bf)
        )
        # w2 stream: [p, ch, o] with ch = kh*32 + ct, value 32*W2[(o,c0+ct), kh*128+p]
        w2s = (W2r[:, c0:c0 + CS, :] * W2SCALE).transpose(2, 1, 0)  # [k, ct, o]
        w2s = w2s.reshape(2, 128, CS, O).transpose(1, 0, 2, 3)      # [p, kh, ct, o]
        w2q = np.ascontiguousarray(w2s.reshape(128, NCHUNK, O)).astype(f8)
        in_maps.append({"pk": pk, "w2": w2q})
    return in_maps


def kernel(x, W1, b1, W2, b2, Wa=None, ba=None, **_unused):
    from concourse.bass_utils import run_bass_kernel_spmd

    if "nc" not in _CACHE:
        _CACHE["nc"] = _build_nc()
    nc = _CACHE["nc"]

    in_maps = _prep_in_maps(x, W1, b1, W2, b2)
    res = run_bass_kernel_spmd(nc, in_maps, core_ids=list(range(NCORES)))
    partials = [r["out"].astype(np.float64) for r in res.results]
    out = np.sum(partials, axis=0).astype(np.float32)
    return out.reshape(B, O, 1, 1)
